# revision 2
# baseline (speedup 1.0000x reference)
"""Expert-parallel MoE FFN kernel for 8 Trainium2 NeuronCores.

Math (per expert e): out = gelu(x_e @ w1_e + b1_e) @ w2_e + b2_e
  x: [B=2, E=8, N=1024, D=1024], w1: [E, D, F=4096], b1: [E, F],
  w2: [E, F, D], b2: [E, D]  ->  out: [B, E, N, D]

Sharding: one expert per core (the e axis); host scatters inputs and
gathers outputs.

Per-core strategy: all matmul operands in bf16 (rel err ~3e-3, well
inside the 2e-2 gate) so BOTH weight matrices stay resident in SBUF
(128 KiB/partition) and are loaded once per dispatch instead of
re-streamed per token block. x is transposed/cast on the host during
the shard scatter, so the device PE issues nothing but the 2048
N=512 matmuls that are this problem's hard roofline (~256 ns each at
the sustained power-limited clock).

Per-core program (TOK=2048 tokens, 4 blocks of TB=512):
  preload: w1res [128,8dc,4096f], w2res [128,32fc,1024d] (bf16),
           b1 [128,32] f32, b2 broadcast [128,1024] f32.
  per block:
    xTt [128,8dc,512] bf16  <- DMA slice of host-provided xT
    mm1: for fg in 32: psum[f128,512tok] = sum_dc w1res^T @ xTt ;
         ACT exact Gelu + b1 -> hT[:, fg, :] (bf16)
    mm2: for ts in 4: two interleaved psum accumulations (d halves,
         sharing each hT stationary load): psum[tok128,512d] =
         sum_fc hT^T @ w2res ; DVE += b2 -> ot f32 ; DMA store.
All matmuls keep a single unified 8-bank PSUM pool so the PE never
waits on bank reuse; ACT/DVE/DMA trail far behind the PE stream.
"""

import sys

for _p in ("/opt/trn_rl_repo", "/opt/pypackages"):
    if _p not in sys.path:
        sys.path.append(_p)

import numpy as np

B, E, N, D, F = 2, 8, 1024, 1024, 4096
TOK = B * N  # tokens per expert
TB = 512  # token block
NBLK = TOK // TB
nD = D // 128
nF = F // 128
nTS = TB // 128

_CACHE: dict = {}


def _build(reps: int = 1):
    import concourse.bacc as bacc
    import concourse.bass as bass
    import concourse.tile as tile
    from concourse import mybir

    F32 = mybir.dt.float32
    BF16 = mybir.dt.bfloat16
    GELU = mybir.ActivationFunctionType.Gelu
    ADD = mybir.AluOpType.add

    nc = bacc.Bacc("TRN2", target_bir_lowering=False, debug=False, num_devices=8)

    xT = nc.dram_tensor("x", [D, TOK], BF16, kind="ExternalInput").ap()
    w1 = nc.dram_tensor("w1", [D, F], BF16, kind="ExternalInput").ap()
    b1 = nc.dram_tensor("b1", [F], F32, kind="ExternalInput").ap()
    w2 = nc.dram_tensor("w2", [F, D], BF16, kind="ExternalInput").ap()
    b2 = nc.dram_tensor("b2", [D], F32, kind="ExternalInput").ap()
    out = nc.dram_tensor("out", [TOK, D], F32, kind="ExternalOutput").ap()

    # multi-dim views for coalesced DMAs
    xT4 = xT.rearrange("(dc p) (blk t) -> blk p dc t", p=128, t=TB)
    w1v = w1.rearrange("(dc p) f -> p dc f", p=128)
    w2v = w2.rearrange("(fc p) d -> p fc d", p=128)
    out4 = out.rearrange("(blk ts p) (dh c) -> blk ts p dh c", ts=nTS, p=128, c=512)

    with tile.TileContext(nc) as tc:
        with (
            tc.tile_pool(name="consts", bufs=1) as consts,
            tc.tile_pool(name="xTp", bufs=2) as xTp,
            tc.tile_pool(name="hTp", bufs=1) as hTp,
            tc.tile_pool(name="op", bufs=2) as op,
            tc.tile_pool(name="ps1", bufs=8, space="PSUM") as ps1,
        ):
            b1_t = consts.tile([128, nF], F32, tag="b1")
            nc.sync.dma_start(out=b1_t, in_=b1.rearrange("(c p) -> p c", p=128))
            b2_t = consts.tile([128, D], F32, tag="b2")
            nc.gpsimd.dma_start(
                out=b2_t,
                in_=bass.AP(tensor=b2.tensor, offset=b2.offset, ap=[[0, 128], [1, D]]),
            )
            # resident weights (bf16): w1 64 KiB/part + w2 64 KiB/part
            w1res = consts.tile([128, nD, F], BF16, tag="w1res")
            for dc in range(nD):
                nc.sync.dma_start(out=w1res[:, dc, :], in_=w1v[:, dc, :])
            w2res = consts.tile([128, nF, D], BF16, tag="w2res")
            for fc in range(nF):
                nc.scalar.dma_start(out=w2res[:, fc, :], in_=w2v[:, fc, :])

            for blk in range(NBLK * reps):
                blk = blk % NBLK

                xTt = xTp.tile([128, nD, TB], BF16, tag="xT")
                nc.sync.dma_start(out=xTt, in_=xT4[blk])

                # --- mm1 + gelu: hT [f, tok] (bf16) ---
                hT_t = hTp.tile([128, nF, TB], BF16, tag="hT")
                for fg in range(nF):
                    ph = ps1.tile([128, TB], F32, tag="ps1", name=f"ph_{blk}_{fg}")
                    for dc in range(nD):
                        nc.tensor.matmul(
                            ph,
                            w1res[:, dc, fg * 128 : (fg + 1) * 128],
                            xTt[:, dc, :],
                            start=(dc == 0),
                            stop=(dc == nD - 1),
                        )
                    nc.scalar.activation(
                        hT_t[:, fg, :], ph, GELU, bias=b1_t[:, fg : fg + 1], scale=1.0
                    )

                # --- mm2 + b2: out [tok, d] ---
                # dh-interleaved: each hT stationary serves both d halves
                for ts in range(nTS):
                    ot = op.tile([128, D], F32, tag="o", name=f"o_{blk}_{ts}")
                    pos = [
                        ps1.tile([128, 512], F32, tag="ps1", name=f"po_{blk}_{ts}_{dh}")
                        for dh in range(D // 512)
                    ]
                    for fc in range(nF):
                        for dh in range(D // 512):
                            nc.tensor.matmul(
                                pos[dh],
                                hT_t[:, fc, ts * 128 : (ts + 1) * 128],
                                w2res[:, fc, dh * 512 : (dh + 1) * 512],
                                start=(fc == 0),
                                stop=(fc == nF - 1),
                            )
                    for dh in range(D // 512):
                        nc.vector.tensor_tensor(
                            out=ot[:, dh * 512 : (dh + 1) * 512],
                            in0=pos[dh],
                            in1=b2_t[:, dh * 512 : (dh + 1) * 512],
                            op=ADD,
                        )
                    nc.scalar.dma_start(
                        out=out4[blk, ts].rearrange("p dh c -> p (dh c)"), in_=ot
                    )

    nc.compile()
    return nc


def _get_nc(reps: int = 1):
    key = f"nc{reps}"
    if key not in _CACHE:
        _CACHE[key] = _build(reps)
    return _CACHE[key]


def make_in_maps(x, w1, b1, w2, b2):
    """Host-side shard scatter: per-expert slices, bf16 cast, x pre-transposed."""
    import ml_dtypes

    bf16 = ml_dtypes.bfloat16
    x = np.asarray(x, dtype=np.float32)
    in_maps = []
    for e in range(E):
        xT_e = np.ascontiguousarray(x[:, e].reshape(TOK, D).T.astype(bf16))
        in_maps.append(
            {
                "x": xT_e,
                "w1": np.ascontiguousarray(np.asarray(w1[e], np.float32).astype(bf16)),
                "b1": np.ascontiguousarray(np.asarray(b1[e], np.float32)),
                "w2": np.ascontiguousarray(np.asarray(w2[e], np.float32).astype(bf16)),
                "b2": np.ascontiguousarray(np.asarray(b2[e], np.float32)),
            }
        )
    return in_maps


def kernel(x, w1, b1, w2, b2):
    from concourse.bass_utils import run_bass_kernel_spmd

    nc = _get_nc()
    in_maps = make_in_maps(x, w1, b1, w2, b2)
    res = run_bass_kernel_spmd(nc, in_maps, list(range(E)))
    out = np.empty((B, E, N, D), np.float32)
    for e in range(E):
        out[:, e] = res.results[e]["out"].reshape(B, N, D)
    return out


# revision 3
# speedup vs baseline: 2.2060x; 2.2060x over previous
"""Expert-parallel MoE FFN kernel for 8 Trainium2 NeuronCores.

Math (per expert e): out = gelu(x_e @ w1_e + b1_e) @ w2_e + b2_e
  x: [B=2, E=8, N=1024, D=1024], w1: [E, D, F=4096], b1: [E, F],
  w2: [E, F, D], b2: [E, D]  ->  out: [B, E, N, D]

Sharding: one expert per core (the e axis); host scatters inputs and
gathers outputs.

Per-core strategy: all matmul operands in bf16 (rel err ~3e-3, well
inside the 2e-2 gate) so BOTH weight matrices stay resident in SBUF
(128 KiB/partition) and are loaded once per dispatch instead of
re-streamed per token block. x is transposed/cast on the host during
the shard scatter, so the device PE issues nothing but the 2048
N=512 matmuls that are this problem's hard roofline (~256 ns each at
the sustained power-limited clock).

Per-core program (TOK=2048 tokens, 4 blocks of TB=512):
  preload: w1res [128,8dc,4096f], w2res [128,32fc,1024d] (bf16),
           b1 [128,32] f32, b2 broadcast [128,1024] f32.
  per block:
    xTt [128,8dc,512] bf16  <- DMA slice of host-provided xT
    mm1: for fg in 32: psum[f128,512tok] = sum_dc w1res^T @ xTt ;
         ACT exact Gelu + b1 -> hT[:, fg, :] (bf16)
    mm2: for ts in 4: two interleaved psum accumulations (d halves,
         sharing each hT stationary load): psum[tok128,512d] =
         sum_fc hT^T @ w2res ; DVE += b2 -> ot f32 ; DMA store.
All matmuls keep a single unified 8-bank PSUM pool so the PE never
waits on bank reuse; ACT/DVE/DMA trail far behind the PE stream.
"""

import sys

for _p in ("/opt/trn_rl_repo", "/opt/pypackages"):
    if _p not in sys.path:
        sys.path.append(_p)

import numpy as np

B, E, N, D, F = 2, 8, 1024, 1024, 4096
TOK = B * N  # tokens per expert
TB = 512  # token block
NBLK = TOK // TB
nD = D // 128
nF = F // 128
nTS = TB // 128

_CACHE: dict = {}


def _build(reps: int = 1):
    import concourse.bacc as bacc
    import concourse.bass as bass
    import concourse.tile as tile
    from concourse import mybir

    F32 = mybir.dt.float32
    BF16 = mybir.dt.bfloat16
    GELU = mybir.ActivationFunctionType.Gelu
    ADD = mybir.AluOpType.add

    nc = bacc.Bacc("TRN2", target_bir_lowering=False, debug=False, num_devices=8)

    xT = nc.dram_tensor("x", [D, TOK], BF16, kind="ExternalInput").ap()
    w1 = nc.dram_tensor("w1", [D, F], BF16, kind="ExternalInput").ap()
    b1 = nc.dram_tensor("b1", [F], F32, kind="ExternalInput").ap()
    w2 = nc.dram_tensor("w2", [F, D], BF16, kind="ExternalInput").ap()
    b2 = nc.dram_tensor("b2", [D], F32, kind="ExternalInput").ap()
    out = nc.dram_tensor("out", [TOK, D], F32, kind="ExternalOutput").ap()

    # multi-dim views for coalesced DMAs
    xT4 = xT.rearrange("(dc p) (blk t) -> blk p dc t", p=128, t=TB)
    w1v = w1.rearrange("(dc p) f -> p dc f", p=128)
    w2v = w2.rearrange("(fc p) d -> p fc d", p=128)
    out4 = out.rearrange("(blk ts p) (dh c) -> blk ts p dh c", ts=nTS, p=128, c=512)

    with tile.TileContext(nc) as tc:
        with (
            tc.tile_pool(name="consts", bufs=1) as consts,
            tc.tile_pool(name="xTp", bufs=2) as xTp,
            tc.tile_pool(name="hTp", bufs=1) as hTp,
            tc.tile_pool(name="op", bufs=2) as op,
            tc.tile_pool(name="ps1", bufs=8, space="PSUM") as ps1,
        ):
            b1_t = consts.tile([128, nF], F32, tag="b1")
            nc.sync.dma_start(out=b1_t, in_=b1.rearrange("(c p) -> p c", p=128))
            b2_t = consts.tile([128, D], F32, tag="b2")
            nc.gpsimd.dma_start(
                out=b2_t,
                in_=bass.AP(tensor=b2.tensor, offset=b2.offset, ap=[[0, 128], [1, D]]),
            )
            # resident weights (bf16): w1 64 KiB/part + w2 64 KiB/part
            w1res = consts.tile([128, nD, F], BF16, tag="w1res")
            for dc in range(nD):
                nc.sync.dma_start(out=w1res[:, dc, :], in_=w1v[:, dc, :])
            w2res = consts.tile([128, nF, D], BF16, tag="w2res")
            for fc in range(nF):
                nc.scalar.dma_start(out=w2res[:, fc, :], in_=w2v[:, fc, :])

            for blk in range(NBLK * reps):
                blk = blk % NBLK

                xTt = xTp.tile([128, nD, TB], BF16, tag="xT")
                nc.sync.dma_start(out=xTt, in_=xT4[blk])

                # --- mm1 + gelu: hT [f, tok] (bf16) ---
                # 4-way interleaved fg groups: amortizes the psum group-start
                # cost over 32 MMs instead of 8
                hT_t = hTp.tile([128, nF, TB], BF16, tag="hT")
                for fg0 in range(0, nF, 4):
                    phs = [
                        ps1.tile([128, TB], F32, tag="ps1", name=f"ph_{blk}_{fg0}_{j}")
                        for j in range(4)
                    ]
                    for dc in range(nD):
                        for j in range(4):
                            nc.tensor.matmul(
                                phs[j],
                                w1res[:, dc, (fg0 + j) * 128 : (fg0 + j + 1) * 128],
                                xTt[:, dc, :],
                                start=(dc == 0),
                                stop=(dc == nD - 1),
                            )
                    for j in range(4):
                        nc.scalar.activation(
                            hT_t[:, fg0 + j, :],
                            phs[j],
                            GELU,
                            bias=b1_t[:, fg0 + j : fg0 + j + 1],
                            scale=1.0,
                        )

                # --- mm2 + b2: out [tok, d] ---
                # dh-interleaved: each hT stationary serves both d halves
                for ts in range(nTS):
                    ot = op.tile([128, D], F32, tag="o", name=f"o_{blk}_{ts}")
                    pos = [
                        ps1.tile([128, 512], F32, tag="ps1", name=f"po_{blk}_{ts}_{dh}")
                        for dh in range(D // 512)
                    ]
                    for fc in range(nF):
                        for dh in range(D // 512):
                            nc.tensor.matmul(
                                pos[dh],
                                hT_t[:, fc, ts * 128 : (ts + 1) * 128],
                                w2res[:, fc, dh * 512 : (dh + 1) * 512],
                                start=(fc == 0),
                                stop=(fc == nF - 1),
                            )
                    for dh in range(D // 512):
                        nc.vector.tensor_tensor(
                            out=ot[:, dh * 512 : (dh + 1) * 512],
                            in0=pos[dh],
                            in1=b2_t[:, dh * 512 : (dh + 1) * 512],
                            op=ADD,
                        )
                    nc.scalar.dma_start(
                        out=out4[blk, ts].rearrange("p dh c -> p (dh c)"), in_=ot
                    )

    nc.compile()
    return nc


def _get_nc(reps: int = 1):
    key = f"nc{reps}"
    if key not in _CACHE:
        _CACHE[key] = _build(reps)
    return _CACHE[key]


def make_in_maps(x, w1, b1, w2, b2):
    """Host-side shard scatter: per-expert slices, bf16 cast, x pre-transposed."""
    import ml_dtypes

    bf16 = ml_dtypes.bfloat16
    x = np.asarray(x, dtype=np.float32)
    in_maps = []
    for e in range(E):
        xT_e = np.ascontiguousarray(x[:, e].reshape(TOK, D).T.astype(bf16))
        in_maps.append(
            {
                "x": xT_e,
                "w1": np.ascontiguousarray(np.asarray(w1[e], np.float32).astype(bf16)),
                "b1": np.ascontiguousarray(np.asarray(b1[e], np.float32)),
                "w2": np.ascontiguousarray(np.asarray(w2[e], np.float32).astype(bf16)),
                "b2": np.ascontiguousarray(np.asarray(b2[e], np.float32)),
            }
        )
    return in_maps


def kernel(x, w1, b1, w2, b2):
    from concourse.bass_utils import run_bass_kernel_spmd

    nc = _get_nc()
    in_maps = make_in_maps(x, w1, b1, w2, b2)
    res = run_bass_kernel_spmd(nc, in_maps, list(range(E)))
    out = np.empty((B, E, N, D), np.float32)
    for e in range(E):
        out[:, e] = res.results[e]["out"].reshape(B, N, D)
    return out
